# revision 30
# baseline (speedup 1.0000x reference)
"""Trainium2 Bass kernel for nn_ATACSplitPool (segment_reduce).

Strategy
--------
The 1.02 GB `x` tensor dominates; it has exactly two consumers:
  1. ragged per-peak segment means (all segment boundaries are multiples of
     PATCH=25 rows for well-formed inputs), and
  2. a MaxPool1d(25) feeding a tiny conv/batchnorm stack.
So the device kernel makes ONE streaming pass over x per core (batch*length
sharded 8 ways: each core gets half of one sample = 50000 rows) computing
  - per-patch sums  (2000, 639)  -> host finishes ragged segment means
  - per-patch maxes (639-major)  -> host finishes the small conv/BN tail
Per core: ~64 MB in (bf16), ~7.7 MB out; everything downstream operates on
<=41 MB and runs on the host in numpy.

Device dataflow per 1000-row group (8 tiles of 125 rows, rows on partitions):
  - PE transposes each 128-channel block of the 8 tiles into one 2KB PSUM
    bank (bf16 pass-through; chunks strided at 128 cols so every write is
    4B-aligned in PSUM). Channel blocks all span 128 rows: starts
    0/128/256/384/511, the last OVERLAPPING block 3 by one channel so no
    ragged-127 special case exists anywhere downstream (host dedups).
  - PE also computes the patch SUMS with a one-hot (125x5) matmul that
    reuses the just-loaded stationary data (5 moving columns ~ free);
    exact f32 PSUM accumulation. The idle ACT engine copies sums to SBUF.
  - DVE computes the patch MAXES as a pairwise tensor_tensor(max) tree:
    InstTensorReduce never qualifies for the DVE 2x_1p fast mode (it was
    the 98%-busy bottleneck at 1x), but elementwise tensor_tensor does
    when every operand is 2-byte and innermost-packed. Level 1 reads the
    PSUM pairs (k 0:13 vs 12:25, overlap is harmless for max) into bf16
    SBUF; levels 2-5 combine 13->7->4->2->1 in SBUF, merged across all 5
    channel blocks into single wide instructions to amortize fixed costs.
    Max of bf16 inputs is exact in bf16.
  Sums stage in f32 (128,5,patches); maxes in bf16. Both flush with big
  strided DMAs; the max plane ships bit-packed (two bf16 per f32 word of
  the single merged output tensor) halving its DMA bytes; the host
  reinterprets. The host transposes back.

The device datapath is bf16: the host rounds x to bf16 once (outside the
timed dispatch), halving HBM DMA traffic and running PE transposes at full
(non-fp32) rate. Patch sums accumulate in f32, so the only precision loss
is the 2^-8 input rounding (measured end-to-end rel err ~9e-3, gate 2e-2).

Per-dispatch operand count is minimized (one input, one merged output, no
partition_id) because on the axon site each XLA buffer adds ~0.1 ms of
per-execution launch overhead, which dominates the device time.
"""

import sys
import numpy as np
import ml_dtypes

if "/opt/trn_rl_repo" not in sys.path:
    sys.path.insert(0, "/opt/trn_rl_repo")

BF16 = ml_dtypes.bfloat16

B, L, D = 4, 100000, 639
PATCH = 25
ATAC_K, JOINT_K, KS = 16, 16, 3
BN_EPS = 1e-5
Lp = L // PATCH                       # 4000
N_CORES = 8
ROWS_PER_CORE = B * L // N_CORES      # 50000
TILE_ROWS = 125                       # 5 patches per tile
GROUP = 8                             # tiles per input DMA / per PSUM bank
NPT = TILE_ROWS // PATCH              # patches per tile (5)

# channel-block starts: all 128 wide; block 4 overlaps block 3 at channel 511
CBS = [0, 128, 256, 384, 511]

F32 = np.float32


# ---------------------------------------------------------------------------
# device kernel
# ---------------------------------------------------------------------------

_ENGINE_CACHE = {}

PM_TILE = 125 * PATCH                 # rows per patch-major tile (3125)


def _build_device_pm(rows, paths=("sum", "max"), repeat=1):
    """Patch-major pipeline: partition p of each load holds ONE whole patch
    (25 consecutive DRAM rows = 31950 contiguous bytes), so every DMA
    descriptor is 32KB (the row-major layout's 1278B descriptors measured
    only ~113 GB/s/core on this site; 32KB descriptors measure ~132).

      - MAX: pure-SBUF pairwise tensor_tensor(max) ladder over the k (free)
        dim, all levels in the DVE 2x_1p fast mode; output lands patch-major
        (patches, D) bf16, exactly the host layout. No PSUM, no ACT copies.
      - SUM: PE transposes each (k, cblk) slice (125 patches x 128 ch) and
        ACCUMULATES the 25 k-slices into one PSUM f32 tile (start/stop
        accumulation flags) = exact f32 patch sums, channel-major. ACT
        drains them to bf16 staging (bf16 sums add <1e-3 relative error:
        the host divides by segment lengths >= 50).
    Channel blocks are all 128 wide; block 4 overlaps block 3 at channel
    511 (host dedups), so nothing downstream is ragged.
    """
    import concourse.bacc as bacc
    import concourse.bass as bass
    import concourse.mybir as mybir
    import concourse.tile as tile
    from concourse import masks

    f32 = mybir.dt.float32
    bf16 = mybir.dt.bfloat16
    MAX = mybir.AluOpType.max

    assert rows % PM_TILE == 0
    n_tiles = rows // PM_TILE          # 16
    patches = rows // PATCH            # 2000
    ppt = PM_TILE // PATCH             # patches per tile (125)

    nc = bacc.Bacc("TRN2", target_bir_lowering=False, debug=False,
                   enable_partition_id=False)

    xs = nc.dram_tensor("xs", (rows, D), bf16, kind="ExternalInput")
    # merged bf16 output: [0 : 5*128*patches] = channel-major patch sums
    # (bf16), then [...:] = patch-major maxes (patches, D)
    sums_sz = 5 * 128 * patches
    out = nc.dram_tensor("outT", (sums_sz + patches * D,), bf16,
                         kind="ExternalOutput")

    xs_v = xs.ap().rearrange("(t p k) d -> t p (k d)", p=ppt, k=PATCH)
    out_sum_v = out.ap()[0:sums_sz].rearrange(
        "(c p t) -> p c t", c=5, p=128)
    out_pm_v = out.ap()[sums_sz:].rearrange(
        "(t p d) -> p t d", p=ppt, d=D)

    FLUSH = sorted({(n_tiles * (q + 1)) // 4 for q in range(4)})

    with tile.TileContext(nc) as tc:
        with (
            tc.tile_pool(name="io", bufs=3) as io_pool,
            tc.tile_pool(name="stage", bufs=1) as stage_pool,
            tc.tile_pool(name="m1p", bufs=2) as m1_pool,
            tc.tile_pool(name="mtp", bufs=1) as mt_pool,
            tc.tile_pool(name="const", bufs=1) as const_pool,
            tc.tile_pool(name="ps_sum", bufs=1,
                         space=bass.MemorySpace.PSUM) as ps_pool,
        ):
            identity = const_pool.tile([128, 128], bf16)
            masks.make_identity(nc, identity[:])

            stage_sum = stage_pool.tile([128, 5, n_tiles, ppt], bf16)
            stage_pm = stage_pool.tile([ppt, n_tiles, D], bf16)

            for m in range(n_tiles * repeat):
                t = m % n_tiles
                xt = io_pool.tile([ppt, PATCH * D], bf16, tag="xt")
                nc.sync.dma_start(xt[:], xs_v[t])
                tv = xt[:].rearrange("p (k d) -> p k d", d=D)

                if "sum" in paths:
                    # each channel block's accumulator padded to one full
                    # 2KB PSUM bank: a multi-matmul accumulation region must
                    # not straddle banks (straddling silently corrupts)
                    ps = ps_pool.tile([128, 5, ppt], f32, tag="ps",
                                      padded_shape=[128, 5, 512])
                    for c in range(5):
                        cs = CBS[c]
                        for k in range(PATCH):
                            # blk.T @ I = transposed k-slice; plain matmul
                            # (not is_transpose) so PSUM accumulates in f32
                            nc.tensor.matmul(
                                ps[:, c, :], tv[:, k, cs:cs + 128],
                                identity[:ppt, :ppt],
                                start=(k == 0), stop=(k == PATCH - 1),
                            )
                    with nc.allow_low_precision(
                            reason="bf16 sums: host divides by len>=50"):
                        nc.scalar.copy(stage_sum[:, :, t, :], ps[:])

                if "max" in paths:
                    m1 = m1_pool.tile([ppt, 13, D], bf16, tag="m1")
                    m2 = mt_pool.tile([ppt, 7, D], bf16, tag="m2")
                    m3 = mt_pool.tile([ppt, 4, D], bf16, tag="m3")
                    m4 = mt_pool.tile([ppt, 2, D], bf16, tag="m4")
                    with nc.allow_low_precision(
                            reason="max of bf16 is exact in bf16"):
                        nc.vector.tensor_tensor(
                            m1[:], tv[:, 0:13, :], tv[:, 12:25, :], op=MAX)
                        nc.vector.tensor_tensor(
                            m2[:], m1[:, 0:7, :], m1[:, 6:13, :], op=MAX)
                        nc.vector.tensor_tensor(
                            m3[:], m2[:, 0:4, :], m2[:, 3:7, :], op=MAX)
                        nc.vector.tensor_tensor(
                            m4[:], m3[:, 0:2, :], m3[:, 2:4, :], op=MAX)
                        nc.vector.tensor_tensor(
                            stage_pm[:, t, :], m4[:, 0, :], m4[:, 1, :],
                            op=MAX)

                if "sum" not in paths and "max" not in paths:
                    nc.scalar.mul(xt[0:32, 0:8], xt[0:32, 0:8], 1.0)

                if t + 1 in FLUSH:
                    t0 = max(ft for ft in [0] + FLUSH if ft < t + 1)
                    ts = slice(t0, t + 1)
                    if "sum" in paths:
                        nc.scalar.dma_start(
                            out_sum_v.rearrange("p c (t q) -> p c t q", q=ppt)
                            [:, :, ts, :],
                            stage_sum[:, :, ts, :],
                        )
                    if "max" in paths:
                        nc.scalar.dma_start(
                            out_pm_v[:, ts, :], stage_pm[:, ts, :])

    nc.compile()
    return nc


def _build_device(rows, paths=("sum", "max"), repeat=1, max_via="tree"):
    """repeat>1 replays the whole compute loop inside one NEFF — used only
    for loop-amplified device-time measurement. The graded path uses
    repeat=1.

    max_via: "tree" = pairwise tensor_tensor(max) ladder (2x_1p DVE mode);
    "reduce" = single InstTensorReduce per block (1x, baseline)."""
    import concourse.bacc as bacc
    import concourse.bass as bass
    import concourse.mybir as mybir
    import concourse.tile as tile
    from concourse import masks

    f32 = mybir.dt.float32
    bf16 = mybir.dt.bfloat16
    X = mybir.AxisListType.X
    MAX = mybir.AluOpType.max

    n_tiles = rows // TILE_ROWS
    assert rows % (TILE_ROWS * GROUP) == 0
    n_groups = n_tiles // GROUP
    patches = rows // PATCH
    npat = TILE_ROWS * GROUP // PATCH  # patches per group (40)

    nc = bacc.Bacc("TRN2", target_bir_lowering=False, debug=False,
                   enable_partition_id=False)

    xs = nc.dram_tensor("xs", (rows, D), bf16, kind="ExternalInput")
    # merged output: [:, :, 0:patches] f32 channel-major patch sums;
    # [:, :, patches:patches+patches//2] = patch maxes, two bf16 packed per
    # f32 word (host reinterprets)
    out = nc.dram_tensor("outT", (5, 128, patches + patches // 2), f32,
                         kind="ExternalOutput")

    xs_v = xs.ap().rearrange("(g i p) d -> g p i d", i=GROUP, p=TILE_ROWS)
    out_sum_v = out.ap()[:, :, 0:patches].rearrange("c p t -> p c t")
    out_max_v = out.ap()[:, :, patches:patches + patches // 2].rearrange(
        "c p t -> p c t")

    # sum staging epochs: one persistent PSUM tile per channel block holds
    # EPOCH groups of one-hot-matmul sums before a single big ACT copy to
    # SBUF (amortizes ACT fixed cost); sum flushes align to epoch ends.
    # sum flushes: 8 points; max flushes: 4 points (wide contiguous runs
    # keep DMA descriptors >=1KB after bf16 packing)
    flush_sum = sorted({(n_groups * (q + 1)) // 8 for q in range(8)})
    flush_max = sorted({(n_groups * (q + 1)) // 4 for q in range(4)})

    with tile.TileContext(nc) as tc:
        with (
            tc.tile_pool(name="io", bufs=6) as io_pool,
            tc.tile_pool(name="stage", bufs=1) as stage_pool,
            tc.tile_pool(name="tree", bufs=3) as tree_pool,
            tc.tile_pool(name="const", bufs=1) as const_pool,
            tc.tile_pool(name="ps_tr", bufs=4,
                         space=bass.MemorySpace.PSUM) as tr_pool,
            tc.tile_pool(name="ps_st", bufs=3,
                         space=bass.MemorySpace.PSUM) as sum_pool,
        ):
            identity = const_pool.tile([128, 128], bf16)
            masks.make_identity(nc, identity[:])
            # a5[p, j] = 1 iff j == p//PATCH: row-sums of identity over
            # 25-col groups (exact 0/1 values in bf16)
            a5_sb = const_pool.tile([TILE_ROWS, NPT], bf16)
            with nc.allow_low_precision(reason="one-hot sums are 0/1"):
                nc.vector.reduce_sum(
                    a5_sb[:],
                    identity[:TILE_ROWS, :TILE_ROWS].rearrange(
                        "p (j k) -> p j k", k=PATCH),
                    axis=X,
                )

            stage_sum = stage_pool.tile([128, 5, patches], f32)
            stage_max = stage_pool.tile([128, 5, patches], bf16)

            xt = None
            for m in range(n_tiles * repeat):
                g, i = (m // GROUP) % n_groups, m % GROUP
                if i == 0:
                    xt = io_pool.tile([TILE_ROWS, GROUP, D], bf16, tag="xt")
                    nc.sync.dma_start(xt[:], xs_v[g])
                if "sum" not in paths and "max" not in paths and i == 0:
                    # keep the DMA consumed so buffer rotation still throttles
                    nc.scalar.mul(xt[0:32, 0, 0:8], xt[0:32, 0, 0:8], 1.0)

                if i != GROUP - 1:
                    continue

                cols = slice(g * npat, (g + 1) * npat)
                t1 = c1 = st = None
                if max_via == "tree" and "max" in paths:
                    t1 = tree_pool.tile([128, 5, GROUP, NPT, 13], bf16,
                                        tag="t1")
                    c1 = tree_pool.tile([128, 5, GROUP, NPT, 13], bf16,
                                        tag="c1")
                if "sum" in paths:
                    st = sum_pool.tile([128, 5, GROUP, NPT], f32, tag="st",
                                       padded_shape=[128, 5, GROUP, 12])
                for c in range(5):
                    cs = CBS[c]
                    tr = tr_pool.tile([128, GROUP * 128], bf16, tag="tr",
                                      padded_shape=[128, GROUP * 128])
                    for j in range(GROUP):
                        blk = xt[:, j, cs:cs + 128]
                        nc.tensor.transpose(
                            tr[:, j * 128:j * 128 + TILE_ROWS],
                            blk, identity[:TILE_ROWS, :TILE_ROWS],
                        )
                        if st is not None:
                            # one-hot matmul reuses the same stationary blk;
                            # 5 cols each, own accumulation group
                            nc.tensor.matmul(st[:, c, j], blk, a5_sb[:])

                    if "max" not in paths:
                        continue
                    trv = tr[:].rearrange(
                        "p (j c2) -> p j c2", c2=128
                    )[:, :, 0:TILE_ROWS].rearrange(
                        "p j (n k) -> p j n k", k=PATCH
                    )
                    if max_via == "tree":
                        # the BIR verifier only allows ONE PSUM operand per
                        # DVE instruction: ACT (otherwise idle) copies the
                        # upper half of each k-range to SBUF, then DVE's
                        # level-1 tensor_tensor pairs PSUM k0:13 with SBUF
                        # k12:25 (overlap at 12 is harmless for max) in the
                        # 2x_1p fast mode
                        nc.scalar.copy(c1[:, c], trv[:, :, :, 12:25])
                        with nc.allow_low_precision(
                                reason="max of bf16 is exact in bf16"):
                            nc.vector.tensor_tensor(
                                t1[:, c], trv[:, :, :, 0:13], c1[:, c],
                                op=MAX)
                    else:
                        dst = stage_max[:, c, cols].rearrange(
                            "p (j n) -> p j n", n=NPT)
                        with nc.allow_low_precision(
                                reason="max of bf16 is exact in bf16"):
                            nc.vector.reduce_max(dst, trv, axis=X)

                if st is not None:
                    # one merged ACT copy drains the group's sums PSUM->SBUF
                    nc.scalar.copy(
                        stage_sum[:, :, cols].rearrange(
                            "p c (j n) -> p c j n", n=NPT),
                        st[:],
                    )

                if max_via == "tree" and "max" in paths:
                    # levels 2-5, merged across all 5 channel blocks (the
                    # Pool/GPSIMD engine cannot run TensorTensor on real HW,
                    # so the whole tail stays on DVE)
                    t2 = tree_pool.tile([128, 5, GROUP, NPT, 7], bf16,
                                        tag="t2")
                    t3 = tree_pool.tile([128, 5, GROUP, NPT, 4], bf16,
                                        tag="t3")
                    t4 = tree_pool.tile([128, 5, GROUP, NPT, 2], bf16,
                                        tag="t4")
                    dst5 = stage_max[:, :, cols].rearrange(
                        "p c (j n) -> p c j n", n=NPT)
                    with nc.allow_low_precision(
                            reason="max of bf16 is exact in bf16"):
                        nc.vector.tensor_tensor(
                            t2[:], t1[:, :, :, :, 0:7], t1[:, :, :, :, 6:13],
                            op=MAX)
                        nc.vector.tensor_tensor(
                            t3[:], t2[:, :, :, :, 0:4], t2[:, :, :, :, 3:7],
                            op=MAX)
                        nc.vector.tensor_tensor(
                            t4[:], t3[:, :, :, :, 0:2], t3[:, :, :, :, 2:4],
                            op=MAX)
                        nc.vector.tensor_tensor(
                            dst5, t4[:, :, :, :, 0], t4[:, :, :, :, 1],
                            op=MAX)

                if "sum" in paths and g + 1 in flush_sum:
                    g0 = max(fg for fg in [0] + flush_sum if fg < g + 1)
                    fcols = slice(g0 * npat, (g + 1) * npat)
                    nc.scalar.dma_start(
                        out_sum_v[:, :, fcols], stage_sum[:, :, fcols]
                    )
                if "max" in paths and g + 1 in flush_max:
                    g0 = max(fg for fg in [0] + flush_max if fg < g + 1)
                    # bf16 pairs pack into f32 words: halve the col range
                    src = stage_max[:, :, g0 * npat:(g + 1) * npat].bitcast(f32)
                    nc.scalar.dma_start(
                        out_max_v[:, :, (g0 * npat) // 2:((g + 1) * npat) // 2],
                        src,
                    )

    nc.compile()
    return nc


def _get_engine(rows=ROWS_PER_CORE, paths=("sum", "max"), design="pm",
                max_via="tree"):
    key = (rows, paths, design, max_via)
    if key not in _ENGINE_CACHE:
        if design == "pm":
            _ENGINE_CACHE[key] = _build_device_pm(rows, paths)
        else:
            _ENGINE_CACHE[key] = _build_device(rows, paths, max_via=max_via)
    return _ENGINE_CACHE[key]


def run_device(x_bf, rows=ROWS_PER_CORE, trace=False, retries=2):
    """x_bf: (N_CORES*rows, D) bfloat16. Returns per-core output dicts and
    the BassKernelResults (for exec_time when trace=True)."""
    import time as _time
    from concourse import bass_utils

    nc = _get_engine(rows)
    in_maps = [
        {"xs": x_bf[c * rows:(c + 1) * rows]} for c in range(N_CORES)
    ]
    last = None
    for attempt in range(retries + 1):
        try:
            return bass_utils.run_bass_kernel_spmd(
                nc, in_maps, core_ids=list(range(N_CORES)), trace=trace
            )
        except Exception as e:  # transient NRT/relay faults: retry
            last = e
            _time.sleep(2.0 * (attempt + 1))
    raise last


# ---------------------------------------------------------------------------
# host tail (everything downstream of the 25x reduction; <=41 MB of data)
# ---------------------------------------------------------------------------

def _relu(v):
    return np.maximum(v, np.float32(0.0))


def _batch_norm(v):
    m = v.mean(axis=(0, 2), keepdims=True, dtype=np.float64)
    var = (v.astype(np.float64) ** 2).mean(axis=(0, 2), keepdims=True) - m ** 2
    return ((v - m) / np.sqrt(var + BN_EPS)).astype(F32)


def _conv1d_same(v, w):
    # v: (B, Cin, T), w: (Cout, Cin, K=3), zero 'SAME' padding
    Bq, Cin, T = v.shape
    Cout, _, K = w.shape
    vp = np.pad(v, ((0, 0), (0, 0), (1, 1)))
    out = np.zeros((Bq, Cout, T), F32)
    for k in range(K):
        vk = vp[:, :, k:k + T].reshape(Bq * 1, Cin, T)
        for b in range(Bq):
            out[b] += w[:, :, k] @ vk[b]
    return out


def _gather_peaks(chunks, n_peaks, max_n_peaks):
    S = chunks.shape[0]
    npk = np.asarray(n_peaks).astype(np.int64)
    starts = np.concatenate([[0], np.cumsum(npk + 1)[:-1]])
    idx = starts[:, None] + np.arange(int(max_n_peaks))
    mask = np.arange(int(max_n_peaks))[None, :] < npk[:, None]
    out = chunks[np.clip(idx, 0, S - 1)]
    return np.where(mask[..., None], out, np.zeros((), chunks.dtype))


def _segment_mean_rows(flat, split, S):
    """Exact replica of reference._segment_mean (row granularity, any split)."""
    T = flat.shape[0]
    bounds = np.cumsum(split.astype(np.int64))
    seg = np.searchsorted(bounds, np.arange(T), side="right")
    valid = seg < S
    sums = np.zeros((S, flat.shape[1]), np.float64)
    np.add.at(sums, seg[valid], flat[valid].astype(np.float64))
    cnt = np.bincount(seg[valid], minlength=S).astype(np.float64)
    return (sums / np.maximum(cnt, 1.0)[:, None]).astype(F32)


def host_finish(ps, pm, atac, atac_w, joint_w, peak_split, n_peaks, max_n_peaks,
                x_flat=None):
    """ps/pm: (B*Lp, D) patch sums / maxes. Returns (B, P, D+16) f32."""
    S = peak_split.shape[0]
    split64 = peak_split.astype(np.int64)
    bounds = np.cumsum(split64)

    # ---- x_region ----
    aligned = (
        bounds[-1] == B * L
        and np.all(split64 >= 0)
        and np.all(bounds % PATCH == 0)
    )
    if aligned:
        pbounds = bounds // PATCH
        csum = np.concatenate(
            [np.zeros((1, D)), np.cumsum(ps.astype(np.float64), axis=0)]
        )
        starts = np.concatenate([[0], pbounds[:-1]])
        seg_sums = csum[pbounds] - csum[starts]
        chunks_x = (seg_sums / np.maximum(split64, 1)[:, None]).astype(F32)
    else:
        assert x_flat is not None
        chunks_x = _segment_mean_rows(x_flat, split64, S)
    x_region = _gather_peaks(chunks_x, n_peaks, max_n_peaks)

    # ---- joint path ----
    xp = pm.reshape(B, Lp, D).transpose(0, 2, 1)            # (B, 639, 4000)
    atac_l = np.log10(atac.astype(F32) + F32(1.0))
    ap0 = atac_l.reshape(B, 1, Lp, PATCH).max(-1)            # (B, 1, 4000)
    ap1 = _relu(_batch_norm(_conv1d_same(ap0, atac_w)))      # (B, 16, 4000)
    joint_in = np.concatenate([xp, ap1], axis=1)             # (B, 655, 4000)
    c2 = _conv1d_same(joint_in, joint_w)
    joint = _relu(_batch_norm(c2)).transpose(0, 2, 1)        # (B, 4000, 16)

    chunks_j = _segment_mean_rows(
        joint.reshape(-1, JOINT_K), split64 // PATCH, S
    )
    joint_region = _gather_peaks(chunks_j, n_peaks, max_n_peaks)
    joint_region = np.log2(joint_region + F32(1.0))
    return np.concatenate([x_region, joint_region], axis=2).astype(F32)


# ---------------------------------------------------------------------------
# entry point
# ---------------------------------------------------------------------------

def _unplanes(planes, patches):
    # planes: (5, 128, patches) channel-major with overlapping block 4
    # (channels 511..638 live in rows 0..127) -> (patches, 639)
    full = np.empty((patches, D), planes.dtype)
    full[:, 0:512] = planes[0:4].transpose(2, 0, 1).reshape(patches, 512)
    full[:, 511:639] = planes[4].T
    return full


def _assemble(res, rows=ROWS_PER_CORE):
    patches = rows // PATCH
    sums_sz = 5 * 128 * patches
    ps_l, pm_l = [], []
    for r in res:
        o = np.asarray(r["outT"])  # (sums_sz + patches*D,) bf16
        sums = o[0:sums_sz].reshape(5, 128, patches).astype(F32)
        pm = o[sums_sz:].reshape(patches, D).astype(F32)
        ps_l.append(_unplanes(sums, patches))
        pm_l.append(pm)
    return np.concatenate(ps_l, axis=0), np.concatenate(pm_l, axis=0)


def kernel(x, atac, atac_w, joint_w, peak_split, n_peaks, max_n_peaks):
    x = np.ascontiguousarray(np.asarray(x, F32))
    atac = np.asarray(atac, F32)
    atac_w = np.asarray(atac_w, F32)
    joint_w = np.asarray(joint_w, F32)
    peak_split = np.asarray(peak_split)
    n_peaks = np.asarray(n_peaks)

    x_flat = x.reshape(B * L, D)
    try:
        res = run_device(np.ascontiguousarray(x_flat.astype(BF16)))
        ps, pm = _assemble(res.results)
    except Exception:
        # device stack unavailable: correct (slow) host fallback
        xb = x_flat.astype(BF16).astype(F32)
        xr = xb.reshape(B * Lp, PATCH, D)
        ps = xr.sum(axis=1, dtype=F32)
        pm = xr.max(axis=1)
    return host_finish(ps, pm, atac, atac_w, joint_w, peak_split, n_peaks,
                       max_n_peaks, x_flat=x_flat)


# revision 44
# speedup vs baseline: 1.0917x; 1.0917x over previous
"""Trainium2 Bass kernel for nn_ATACSplitPool (segment_reduce).

Strategy
--------
The 1.02 GB `x` tensor dominates; it has exactly two consumers:
  1. ragged per-peak segment means (all segment boundaries are multiples of
     PATCH=25 rows for well-formed inputs), and
  2. a MaxPool1d(25) feeding a tiny conv/batchnorm stack.
So the device kernel makes ONE streaming pass over x per core (batch*length
sharded 8 ways: each core gets half of one sample = 50000 rows) computing
  - per-patch sums  (2000, 639)  -> host finishes ragged segment means
  - per-patch maxes (639-major)  -> host finishes the small conv/BN tail
Per core: ~64 MB in (bf16), ~7.7 MB out; everything downstream operates on
<=41 MB and runs on the host in numpy.

Device dataflow per 1000-row group (8 tiles of 125 rows, rows on partitions):
  - PE transposes each 128-channel block of the 8 tiles into one 2KB PSUM
    bank (bf16 pass-through; chunks strided at 128 cols so every write is
    4B-aligned in PSUM). Channel blocks all span 128 rows: starts
    0/128/256/384/511, the last OVERLAPPING block 3 by one channel so no
    ragged-127 special case exists anywhere downstream (host dedups).
  - PE also computes the patch SUMS with a one-hot (125x5) matmul that
    reuses the just-loaded stationary data (5 moving columns ~ free);
    exact f32 PSUM accumulation. The idle ACT engine copies sums to SBUF.
  - DVE computes the patch MAXES as a pairwise tensor_tensor(max) tree:
    InstTensorReduce never qualifies for the DVE 2x_1p fast mode (it was
    the 98%-busy bottleneck at 1x), but elementwise tensor_tensor does
    when every operand is 2-byte and innermost-packed. Level 1 reads the
    PSUM pairs (k 0:13 vs 12:25, overlap is harmless for max) into bf16
    SBUF; levels 2-5 combine 13->7->4->2->1 in SBUF, merged across all 5
    channel blocks into single wide instructions to amortize fixed costs.
    Max of bf16 inputs is exact in bf16.
  Sums stage in f32 (128,5,patches); maxes in bf16. Both flush with big
  strided DMAs; the max plane ships bit-packed (two bf16 per f32 word of
  the single merged output tensor) halving its DMA bytes; the host
  reinterprets. The host transposes back.

The device datapath is bf16: the host rounds x to bf16 once (outside the
timed dispatch), halving HBM DMA traffic and running PE transposes at full
(non-fp32) rate. Patch sums accumulate in f32, so the only precision loss
is the 2^-8 input rounding (measured end-to-end rel err ~9e-3, gate 2e-2).

Per-dispatch operand count is minimized (one input, one merged output, no
partition_id) because on the axon site each XLA buffer adds ~0.1 ms of
per-execution launch overhead, which dominates the device time.
"""

import sys
import numpy as np
import ml_dtypes

if "/opt/trn_rl_repo" not in sys.path:
    sys.path.insert(0, "/opt/trn_rl_repo")

BF16 = ml_dtypes.bfloat16

B, L, D = 4, 100000, 639
PATCH = 25
ATAC_K, JOINT_K, KS = 16, 16, 3
BN_EPS = 1e-5
Lp = L // PATCH                       # 4000
N_CORES = 8
ROWS_PER_CORE = B * L // N_CORES      # 50000
TILE_ROWS = 125                       # 5 patches per tile
GROUP = 8                             # tiles per input DMA / per PSUM bank
NPT = TILE_ROWS // PATCH              # patches per tile (5)

# channel-block starts: all 128 wide; block 4 overlaps block 3 at channel 511
CBS = [0, 128, 256, 384, 511]

F32 = np.float32


# ---------------------------------------------------------------------------
# device kernel
# ---------------------------------------------------------------------------

_ENGINE_CACHE = {}

PM_TILE = 125 * PATCH                 # rows per patch-major tile (3125)


def _build_device_pm(rows, paths=("sum", "max"), repeat=1, max_via="reduce"):
    """Patch-major pipeline: partition p of each load holds ONE whole patch
    (25 consecutive DRAM rows = 31950 contiguous bytes), so every DMA
    descriptor is 32KB (the row-major layout's 1278B descriptors measured
    only ~113 GB/s/core on this site; 32KB descriptors measure ~132).

      - MAX: pure-SBUF pairwise tensor_tensor(max) ladder over the k (free)
        dim, all levels in the DVE 2x_1p fast mode; output lands patch-major
        (patches, D) bf16, exactly the host layout. No PSUM, no ACT copies.
      - SUM: PE transposes each (k, cblk) slice (125 patches x 128 ch) and
        ACCUMULATES the 25 k-slices into one PSUM f32 tile (start/stop
        accumulation flags) = exact f32 patch sums, channel-major. ACT
        drains them to bf16 staging (bf16 sums add <1e-3 relative error:
        the host divides by segment lengths >= 50).
    Channel blocks are all 128 wide; block 4 overlaps block 3 at channel
    511 (host dedups), so nothing downstream is ragged.
    """
    import concourse.bacc as bacc
    import concourse.bass as bass
    import concourse.mybir as mybir
    import concourse.tile as tile
    from concourse import masks

    f32 = mybir.dt.float32
    bf16 = mybir.dt.bfloat16
    MAX = mybir.AluOpType.max

    assert rows % PM_TILE == 0
    n_tiles = rows // PM_TILE          # 16
    patches = rows // PATCH            # 2000
    ppt = PM_TILE // PATCH             # patches per tile (125)
    DSUM = 640                         # padded channel count for sums plane

    nc = bacc.Bacc("TRN2", target_bir_lowering=False, debug=False,
                   enable_partition_id=False)

    xs = nc.dram_tensor("xs", (rows, D), bf16, kind="ExternalInput")
    # merged bf16 output, both planes patch-major:
    # [0 : patches*640] = patch sums (col 639 zero-pad), then maxes
    sums_sz = patches * DSUM
    out = nc.dram_tensor("outT", (sums_sz + patches * D,), bf16,
                         kind="ExternalOutput")

    xs_v = xs.ap().rearrange("(t p k) d -> t p (k d)", p=ppt, k=PATCH)
    out_sum_v = out.ap()[0:sums_sz].rearrange(
        "(t p d) -> p t d", p=ppt, d=DSUM)
    out_pm_v = out.ap()[sums_sz:].rearrange(
        "(t p d) -> p t d", p=ppt, d=D)

    # channel blocks for the sum matmuls: the moving operand can span up to
    # 512 columns (one full f32 PSUM bank), so 2 matmuls cover all 639
    CBLK = [(0, 512), (512, 127)]

    FLUSH = sorted({(n_tiles * (q + 1)) // 4 for q in range(4)})

    with tile.TileContext(nc) as tc:
        with (
            tc.tile_pool(name="io", bufs=3) as io_pool,
            tc.tile_pool(name="stage", bufs=1) as stage_pool,
            tc.tile_pool(name="m1p", bufs=2) as m1_pool,
            tc.tile_pool(name="mtp", bufs=1) as mt_pool,
            tc.tile_pool(name="const", bufs=1) as const_pool,
            tc.tile_pool(name="ps_sum", bufs=1,
                         space=bass.MemorySpace.PSUM) as ps_pool,
        ):
            identity = const_pool.tile([128, 128], bf16)
            masks.make_identity(nc, identity[:])

            stage_sum = stage_pool.tile([ppt, n_tiles, DSUM], bf16)
            stage_pm = stage_pool.tile([ppt, n_tiles, D], bf16)
            # host drops col 639 of the sums plane; keep it finite
            nc.gpsimd.memset(stage_sum[0:96, :, D:DSUM], 0.0)
            nc.gpsimd.memset(stage_sum[96:ppt, :, D:DSUM], 0.0)

            for m in range(n_tiles * repeat):
                t = m % n_tiles
                xt = io_pool.tile([ppt, PATCH * D], bf16, tag="xt")
                nc.sync.dma_start(xt[:], xs_v[t])
                tv = xt[:].rearrange("p (k d) -> p k d", d=D)

                if "sum" in paths:
                    # identity is the STATIONARY operand (loaded once ever);
                    # the data streams as the moving operand: I.T @ blk_k =
                    # blk_k, PSUM-accumulated over the 25 k-slices = exact
                    # f32 patch sums, already patch-major. Each 512-col
                    # accumulator region is one full 2KB PSUM bank: a
                    # multi-matmul accumulation region must not straddle
                    # banks (straddling silently corrupts).
                    ps = ps_pool.tile([ppt, 2, 512], f32, tag="ps",
                                      padded_shape=[ppt, 2, 512])
                    for c, (cs, w) in enumerate(CBLK):
                        for k in range(PATCH):
                            nc.tensor.matmul(
                                ps[:, c, 0:w], identity[:ppt, :ppt],
                                tv[:, k, cs:cs + w],
                                start=(k == 0), stop=(k == PATCH - 1),
                            )
                    with nc.allow_low_precision(
                            reason="bf16 sums: host divides by len>=50"):
                        for c, (cs, w) in enumerate(CBLK):
                            nc.scalar.copy(
                                stage_sum[:, t, cs:cs + w], ps[:, c, 0:w])

                if "max" in paths:
                    with nc.allow_low_precision(
                            reason="max of bf16 is exact in bf16"):
                        if max_via == "reduce":
                            # one pass, minimal SBUF traffic (reads the tile
                            # once); 1x DVE rate but far under the DMA floor
                            nc.vector.reduce_max(
                                stage_pm[:, t, :],
                                tv.rearrange("p k d -> p d k"),
                                axis=mybir.AxisListType.X)
                        else:
                            m1 = m1_pool.tile([ppt, 13, D], bf16, tag="m1")
                            m2 = mt_pool.tile([ppt, 7, D], bf16, tag="m2")
                            m3 = mt_pool.tile([ppt, 4, D], bf16, tag="m3")
                            m4 = mt_pool.tile([ppt, 2, D], bf16, tag="m4")
                            nc.vector.tensor_tensor(
                                m1[:], tv[:, 0:13, :], tv[:, 12:25, :],
                                op=MAX)
                            nc.vector.tensor_tensor(
                                m2[:], m1[:, 0:7, :], m1[:, 6:13, :], op=MAX)
                            nc.vector.tensor_tensor(
                                m3[:], m2[:, 0:4, :], m2[:, 3:7, :], op=MAX)
                            nc.vector.tensor_tensor(
                                m4[:], m3[:, 0:2, :], m3[:, 2:4, :], op=MAX)
                            nc.vector.tensor_tensor(
                                stage_pm[:, t, :], m4[:, 0, :], m4[:, 1, :],
                                op=MAX)

                if "sum" not in paths and "max" not in paths:
                    nc.scalar.mul(xt[0:32, 0:8], xt[0:32, 0:8], 1.0)

                if t + 1 in FLUSH:
                    t0 = max(ft for ft in [0] + FLUSH if ft < t + 1)
                    ts = slice(t0, t + 1)
                    if "sum" in paths:
                        nc.scalar.dma_start(
                            out_sum_v[:, ts, :], stage_sum[:, ts, :])
                    if "max" in paths:
                        nc.scalar.dma_start(
                            out_pm_v[:, ts, :], stage_pm[:, ts, :])

    nc.compile()
    return nc


def _build_device(rows, paths=("sum", "max"), repeat=1, max_via="tree"):
    """repeat>1 replays the whole compute loop inside one NEFF — used only
    for loop-amplified device-time measurement. The graded path uses
    repeat=1.

    max_via: "tree" = pairwise tensor_tensor(max) ladder (2x_1p DVE mode);
    "reduce" = single InstTensorReduce per block (1x, baseline)."""
    import concourse.bacc as bacc
    import concourse.bass as bass
    import concourse.mybir as mybir
    import concourse.tile as tile
    from concourse import masks

    f32 = mybir.dt.float32
    bf16 = mybir.dt.bfloat16
    X = mybir.AxisListType.X
    MAX = mybir.AluOpType.max

    n_tiles = rows // TILE_ROWS
    assert rows % (TILE_ROWS * GROUP) == 0
    n_groups = n_tiles // GROUP
    patches = rows // PATCH
    npat = TILE_ROWS * GROUP // PATCH  # patches per group (40)

    nc = bacc.Bacc("TRN2", target_bir_lowering=False, debug=False,
                   enable_partition_id=False)

    xs = nc.dram_tensor("xs", (rows, D), bf16, kind="ExternalInput")
    # merged output: [:, :, 0:patches] f32 channel-major patch sums;
    # [:, :, patches:patches+patches//2] = patch maxes, two bf16 packed per
    # f32 word (host reinterprets)
    out = nc.dram_tensor("outT", (5, 128, patches + patches // 2), f32,
                         kind="ExternalOutput")

    xs_v = xs.ap().rearrange("(g i p) d -> g p i d", i=GROUP, p=TILE_ROWS)
    out_sum_v = out.ap()[:, :, 0:patches].rearrange("c p t -> p c t")
    out_max_v = out.ap()[:, :, patches:patches + patches // 2].rearrange(
        "c p t -> p c t")

    # sum staging epochs: one persistent PSUM tile per channel block holds
    # EPOCH groups of one-hot-matmul sums before a single big ACT copy to
    # SBUF (amortizes ACT fixed cost); sum flushes align to epoch ends.
    # sum flushes: 8 points; max flushes: 4 points (wide contiguous runs
    # keep DMA descriptors >=1KB after bf16 packing)
    flush_sum = sorted({(n_groups * (q + 1)) // 8 for q in range(8)})
    flush_max = sorted({(n_groups * (q + 1)) // 4 for q in range(4)})

    with tile.TileContext(nc) as tc:
        with (
            tc.tile_pool(name="io", bufs=6) as io_pool,
            tc.tile_pool(name="stage", bufs=1) as stage_pool,
            tc.tile_pool(name="tree", bufs=3) as tree_pool,
            tc.tile_pool(name="const", bufs=1) as const_pool,
            tc.tile_pool(name="ps_tr", bufs=4,
                         space=bass.MemorySpace.PSUM) as tr_pool,
            tc.tile_pool(name="ps_st", bufs=3,
                         space=bass.MemorySpace.PSUM) as sum_pool,
        ):
            identity = const_pool.tile([128, 128], bf16)
            masks.make_identity(nc, identity[:])
            # a5[p, j] = 1 iff j == p//PATCH: row-sums of identity over
            # 25-col groups (exact 0/1 values in bf16)
            a5_sb = const_pool.tile([TILE_ROWS, NPT], bf16)
            with nc.allow_low_precision(reason="one-hot sums are 0/1"):
                nc.vector.reduce_sum(
                    a5_sb[:],
                    identity[:TILE_ROWS, :TILE_ROWS].rearrange(
                        "p (j k) -> p j k", k=PATCH),
                    axis=X,
                )

            stage_sum = stage_pool.tile([128, 5, patches], f32)
            stage_max = stage_pool.tile([128, 5, patches], bf16)

            xt = None
            for m in range(n_tiles * repeat):
                g, i = (m // GROUP) % n_groups, m % GROUP
                if i == 0:
                    xt = io_pool.tile([TILE_ROWS, GROUP, D], bf16, tag="xt")
                    nc.sync.dma_start(xt[:], xs_v[g])
                if "sum" not in paths and "max" not in paths and i == 0:
                    # keep the DMA consumed so buffer rotation still throttles
                    nc.scalar.mul(xt[0:32, 0, 0:8], xt[0:32, 0, 0:8], 1.0)

                if i != GROUP - 1:
                    continue

                cols = slice(g * npat, (g + 1) * npat)
                t1 = c1 = st = None
                if max_via == "tree" and "max" in paths:
                    t1 = tree_pool.tile([128, 5, GROUP, NPT, 13], bf16,
                                        tag="t1")
                    c1 = tree_pool.tile([128, 5, GROUP, NPT, 13], bf16,
                                        tag="c1")
                if "sum" in paths:
                    st = sum_pool.tile([128, 5, GROUP, NPT], f32, tag="st",
                                       padded_shape=[128, 5, GROUP, 12])
                for c in range(5):
                    cs = CBS[c]
                    tr = tr_pool.tile([128, GROUP * 128], bf16, tag="tr",
                                      padded_shape=[128, GROUP * 128])
                    for j in range(GROUP):
                        blk = xt[:, j, cs:cs + 128]
                        nc.tensor.transpose(
                            tr[:, j * 128:j * 128 + TILE_ROWS],
                            blk, identity[:TILE_ROWS, :TILE_ROWS],
                        )
                        if st is not None:
                            # one-hot matmul reuses the same stationary blk;
                            # 5 cols each, own accumulation group
                            nc.tensor.matmul(st[:, c, j], blk, a5_sb[:])

                    if "max" not in paths:
                        continue
                    trv = tr[:].rearrange(
                        "p (j c2) -> p j c2", c2=128
                    )[:, :, 0:TILE_ROWS].rearrange(
                        "p j (n k) -> p j n k", k=PATCH
                    )
                    if max_via == "tree":
                        # the BIR verifier only allows ONE PSUM operand per
                        # DVE instruction: ACT (otherwise idle) copies the
                        # upper half of each k-range to SBUF, then DVE's
                        # level-1 tensor_tensor pairs PSUM k0:13 with SBUF
                        # k12:25 (overlap at 12 is harmless for max) in the
                        # 2x_1p fast mode
                        nc.scalar.copy(c1[:, c], trv[:, :, :, 12:25])
                        with nc.allow_low_precision(
                                reason="max of bf16 is exact in bf16"):
                            nc.vector.tensor_tensor(
                                t1[:, c], trv[:, :, :, 0:13], c1[:, c],
                                op=MAX)
                    else:
                        dst = stage_max[:, c, cols].rearrange(
                            "p (j n) -> p j n", n=NPT)
                        with nc.allow_low_precision(
                                reason="max of bf16 is exact in bf16"):
                            nc.vector.reduce_max(dst, trv, axis=X)

                if st is not None:
                    # one merged ACT copy drains the group's sums PSUM->SBUF
                    nc.scalar.copy(
                        stage_sum[:, :, cols].rearrange(
                            "p c (j n) -> p c j n", n=NPT),
                        st[:],
                    )

                if max_via == "tree" and "max" in paths:
                    # levels 2-5, merged across all 5 channel blocks (the
                    # Pool/GPSIMD engine cannot run TensorTensor on real HW,
                    # so the whole tail stays on DVE)
                    t2 = tree_pool.tile([128, 5, GROUP, NPT, 7], bf16,
                                        tag="t2")
                    t3 = tree_pool.tile([128, 5, GROUP, NPT, 4], bf16,
                                        tag="t3")
                    t4 = tree_pool.tile([128, 5, GROUP, NPT, 2], bf16,
                                        tag="t4")
                    dst5 = stage_max[:, :, cols].rearrange(
                        "p c (j n) -> p c j n", n=NPT)
                    with nc.allow_low_precision(
                            reason="max of bf16 is exact in bf16"):
                        nc.vector.tensor_tensor(
                            t2[:], t1[:, :, :, :, 0:7], t1[:, :, :, :, 6:13],
                            op=MAX)
                        nc.vector.tensor_tensor(
                            t3[:], t2[:, :, :, :, 0:4], t2[:, :, :, :, 3:7],
                            op=MAX)
                        nc.vector.tensor_tensor(
                            t4[:], t3[:, :, :, :, 0:2], t3[:, :, :, :, 2:4],
                            op=MAX)
                        nc.vector.tensor_tensor(
                            dst5, t4[:, :, :, :, 0], t4[:, :, :, :, 1],
                            op=MAX)

                if "sum" in paths and g + 1 in flush_sum:
                    g0 = max(fg for fg in [0] + flush_sum if fg < g + 1)
                    fcols = slice(g0 * npat, (g + 1) * npat)
                    nc.scalar.dma_start(
                        out_sum_v[:, :, fcols], stage_sum[:, :, fcols]
                    )
                if "max" in paths and g + 1 in flush_max:
                    g0 = max(fg for fg in [0] + flush_max if fg < g + 1)
                    # bf16 pairs pack into f32 words: halve the col range
                    src = stage_max[:, :, g0 * npat:(g + 1) * npat].bitcast(f32)
                    nc.scalar.dma_start(
                        out_max_v[:, :, (g0 * npat) // 2:((g + 1) * npat) // 2],
                        src,
                    )

    nc.compile()
    return nc


def _get_engine(rows=ROWS_PER_CORE, paths=("sum", "max"), design="pm",
                max_via="reduce"):
    key = (rows, paths, design, max_via)
    if key not in _ENGINE_CACHE:
        if design == "pm":
            _ENGINE_CACHE[key] = _build_device_pm(rows, paths,
                                                  max_via=max_via)
        else:
            _ENGINE_CACHE[key] = _build_device(rows, paths, max_via=max_via)
    return _ENGINE_CACHE[key]


def run_device(x_bf, rows=ROWS_PER_CORE, trace=False, retries=2):
    """x_bf: (N_CORES*rows, D) bfloat16. Returns per-core output dicts and
    the BassKernelResults (for exec_time when trace=True)."""
    import time as _time
    from concourse import bass_utils

    nc = _get_engine(rows)
    in_maps = [
        {"xs": x_bf[c * rows:(c + 1) * rows]} for c in range(N_CORES)
    ]
    last = None
    for attempt in range(retries + 1):
        try:
            return bass_utils.run_bass_kernel_spmd(
                nc, in_maps, core_ids=list(range(N_CORES)), trace=trace
            )
        except Exception as e:  # transient NRT/relay faults: retry
            last = e
            _time.sleep(2.0 * (attempt + 1))
    raise last


# ---------------------------------------------------------------------------
# host tail (everything downstream of the 25x reduction; <=41 MB of data)
# ---------------------------------------------------------------------------

def _relu(v):
    return np.maximum(v, np.float32(0.0))


def _batch_norm(v):
    m = v.mean(axis=(0, 2), keepdims=True, dtype=np.float64)
    var = (v.astype(np.float64) ** 2).mean(axis=(0, 2), keepdims=True) - m ** 2
    return ((v - m) / np.sqrt(var + BN_EPS)).astype(F32)


def _conv1d_same(v, w):
    # v: (B, Cin, T), w: (Cout, Cin, K=3), zero 'SAME' padding
    Bq, Cin, T = v.shape
    Cout, _, K = w.shape
    vp = np.pad(v, ((0, 0), (0, 0), (1, 1)))
    out = np.zeros((Bq, Cout, T), F32)
    for k in range(K):
        vk = vp[:, :, k:k + T].reshape(Bq * 1, Cin, T)
        for b in range(Bq):
            out[b] += w[:, :, k] @ vk[b]
    return out


def _gather_peaks(chunks, n_peaks, max_n_peaks):
    S = chunks.shape[0]
    npk = np.asarray(n_peaks).astype(np.int64)
    starts = np.concatenate([[0], np.cumsum(npk + 1)[:-1]])
    idx = starts[:, None] + np.arange(int(max_n_peaks))
    mask = np.arange(int(max_n_peaks))[None, :] < npk[:, None]
    out = chunks[np.clip(idx, 0, S - 1)]
    return np.where(mask[..., None], out, np.zeros((), chunks.dtype))


def _segment_mean_rows(flat, split, S):
    """Exact replica of reference._segment_mean (row granularity, any split)."""
    T = flat.shape[0]
    bounds = np.cumsum(split.astype(np.int64))
    seg = np.searchsorted(bounds, np.arange(T), side="right")
    valid = seg < S
    sums = np.zeros((S, flat.shape[1]), np.float64)
    np.add.at(sums, seg[valid], flat[valid].astype(np.float64))
    cnt = np.bincount(seg[valid], minlength=S).astype(np.float64)
    return (sums / np.maximum(cnt, 1.0)[:, None]).astype(F32)


def host_finish(ps, pm, atac, atac_w, joint_w, peak_split, n_peaks, max_n_peaks,
                x_flat=None):
    """ps/pm: (B*Lp, D) patch sums / maxes. Returns (B, P, D+16) f32."""
    S = peak_split.shape[0]
    split64 = peak_split.astype(np.int64)
    bounds = np.cumsum(split64)

    # ---- x_region ----
    aligned = (
        bounds[-1] == B * L
        and np.all(split64 >= 0)
        and np.all(bounds % PATCH == 0)
    )
    if aligned:
        pbounds = bounds // PATCH
        csum = np.concatenate(
            [np.zeros((1, D)), np.cumsum(ps.astype(np.float64), axis=0)]
        )
        starts = np.concatenate([[0], pbounds[:-1]])
        seg_sums = csum[pbounds] - csum[starts]
        chunks_x = (seg_sums / np.maximum(split64, 1)[:, None]).astype(F32)
    else:
        assert x_flat is not None
        chunks_x = _segment_mean_rows(x_flat, split64, S)
    x_region = _gather_peaks(chunks_x, n_peaks, max_n_peaks)

    # ---- joint path ----
    xp = pm.reshape(B, Lp, D).transpose(0, 2, 1)            # (B, 639, 4000)
    atac_l = np.log10(atac.astype(F32) + F32(1.0))
    ap0 = atac_l.reshape(B, 1, Lp, PATCH).max(-1)            # (B, 1, 4000)
    ap1 = _relu(_batch_norm(_conv1d_same(ap0, atac_w)))      # (B, 16, 4000)
    joint_in = np.concatenate([xp, ap1], axis=1)             # (B, 655, 4000)
    c2 = _conv1d_same(joint_in, joint_w)
    joint = _relu(_batch_norm(c2)).transpose(0, 2, 1)        # (B, 4000, 16)

    chunks_j = _segment_mean_rows(
        joint.reshape(-1, JOINT_K), split64 // PATCH, S
    )
    joint_region = _gather_peaks(chunks_j, n_peaks, max_n_peaks)
    joint_region = np.log2(joint_region + F32(1.0))
    return np.concatenate([x_region, joint_region], axis=2).astype(F32)


# ---------------------------------------------------------------------------
# entry point
# ---------------------------------------------------------------------------

def _unplanes(planes, patches):
    # planes: (5, 128, patches) channel-major with overlapping block 4
    # (channels 511..638 live in rows 0..127) -> (patches, 639)
    full = np.empty((patches, D), planes.dtype)
    full[:, 0:512] = planes[0:4].transpose(2, 0, 1).reshape(patches, 512)
    full[:, 511:639] = planes[4].T
    return full


def _assemble(res, rows=ROWS_PER_CORE):
    patches = rows // PATCH
    sums_sz = patches * 640
    ps_l, pm_l = [], []
    for r in res:
        o = np.asarray(r["outT"])  # (patches*640 + patches*D,) bf16
        ps_l.append(o[0:sums_sz].reshape(patches, 640)[:, :D].astype(F32))
        pm_l.append(o[sums_sz:].reshape(patches, D).astype(F32))
    return np.concatenate(ps_l, axis=0), np.concatenate(pm_l, axis=0)


def kernel(x, atac, atac_w, joint_w, peak_split, n_peaks, max_n_peaks):
    x = np.ascontiguousarray(np.asarray(x, F32))
    atac = np.asarray(atac, F32)
    atac_w = np.asarray(atac_w, F32)
    joint_w = np.asarray(joint_w, F32)
    peak_split = np.asarray(peak_split)
    n_peaks = np.asarray(n_peaks)

    x_flat = x.reshape(B * L, D)
    try:
        res = run_device(np.ascontiguousarray(x_flat.astype(BF16)))
        ps, pm = _assemble(res.results)
    except Exception:
        # device stack unavailable: correct (slow) host fallback
        xb = x_flat.astype(BF16).astype(F32)
        xr = xb.reshape(B * Lp, PATCH, D)
        ps = xr.sum(axis=1, dtype=F32)
        pm = xr.max(axis=1)
    return host_finish(ps, pm, atac, atac_w, joint_w, peak_split, n_peaks,
                       max_n_peaks, x_flat=x_flat)
